# revision 2
# baseline (speedup 1.0000x reference)
"""Trainium2 Bass kernel for the Blurkernel problem.

Computes blur_kernel[1,1,K,K] = normalize(exp(-x^2/(2 s1^2)
- 2 rho x y/(2 s1 s2) - y^2/(2 s2^2))) for K=511 on TRN2 NeuronCores.

Strategy: the grid is K x K with coordinates -R..R (R=K//2).  With
rho == 0 (the case the harness generates) the Gaussian separates:
e[y, x] = exp(c*y^2) * exp(a*x^2), and the global sum factors as
Sy * Sx.  Each core computes, fully on-device:
  - iota x coords [128, K] (same values on every partition)
  - ex = Exp(a*x^2) with free-dim accum -> Sx per partition (all equal)
  - eyrow = Exp(c*x^2) with accum -> Sy per partition (all equal)
  - its own 128 rows' ey from a per-core y-coord input [128,1]
  - out = ex * (ey / (Sx*Sy))  -> one [128, K] tile, DMA'd out
Cores 0..ntiles-1 cover distinct row blocks; the host stacks them.
A general (rho != 0) path computes the full-grid row sums on every
core via iota y tiles and a cross-partition reduce.
"""

import math
import sys
import types

import numpy as np

N_CORES = 8
P = 128


def _install_ntff_shim():
    """Make run_bass_kernel_spmd(trace=True) under axon degrade gracefully
    (or work, when the axon .so supports it) even though this image's
    antenv package lacks the axon_hooks module."""
    if "antenv.axon_hooks" in sys.modules:
        return
    try:
        import antenv.axon_hooks  # noqa: F401
        return
    except ImportError:
        pass
    hook = None
    try:
        from trn_agent_boot.trn_boot import _ntff_profile_via_ctypes

        hook = _ntff_profile_via_ctypes("/opt/axon/libaxon_pjrt.so")
    except Exception:
        hook = None
    mod = types.ModuleType("antenv.axon_hooks")
    mod.get_axon_ntff_profile_hook = lambda: hook
    sys.modules["antenv.axon_hooks"] = mod


def _build(a, c, b, K, ntiles, use_rho):
    """Trace and compile the Bass kernel. a, c, b are f32 immediates."""
    import concourse.bacc as bacc
    import concourse.mybir as mybir
    import concourse.tile as tile

    R = K // 2
    F = mybir.dt.float32
    EXP = mybir.ActivationFunctionType.Exp

    nc = bacc.Bacc(
        "TRN2", target_bir_lowering=False, debug=False, num_devices=N_CORES
    )
    ycoord = nc.dram_tensor("ycoord", [P, 1], F, kind="ExternalInput")
    out = nc.dram_tensor("out", [P, K], F, kind="ExternalOutput")

    with tile.TileContext(nc) as tc:
        with tc.tile_pool(name="pool", bufs=1) as pool:
            # x coordinates -R..R along the free dim, same in every partition
            xi = pool.tile([P, K], F)
            nc.gpsimd.iota(
                xi[:], [[1, K]], base=-R, channel_multiplier=0,
                allow_small_or_imprecise_dtypes=True,
            )
            xsq = pool.tile([P, K], F)
            nc.vector.tensor_mul(xsq[:], xi[:], xi[:])

            # this core's 128 y coords (data differs per core)
            yc = pool.tile([P, 1], F)
            nc.sync.dma_start(yc[:], ycoord[:, :])
            ysq = pool.tile([P, 1], F)
            nc.vector.tensor_mul(ysq[:], yc[:], yc[:])

            # ex = exp(a*x^2), row sum -> Sx (identical on every partition)
            ex = pool.tile([P, K], F)
            sx = pool.tile([P, 1], F)
            nc.scalar.activation(ex[:], xsq[:], EXP, scale=a, accum_out=sx[:])

            if not use_rho:
                # Sy via exp(c*x^2) over the same coord set (free-dim sum
                # -> broadcast across partitions for free)
                eyr = pool.tile([P, K], F)
                sy = pool.tile([P, 1], F)
                nc.scalar.activation(
                    eyr[:], xsq[:], EXP, scale=c, accum_out=sy[:]
                )
                eyc = pool.tile([P, 1], F)
                nc.scalar.activation(eyc[:], ysq[:], EXP, scale=c)

                s = pool.tile([P, 1], F)
                nc.vector.tensor_mul(s[:], sx[:], sy[:])
                inv = pool.tile([P, 1], F)
                nc.vector.reciprocal(inv[:], s[:])
                nrm = pool.tile([P, 1], F)
                nc.vector.tensor_mul(nrm[:], eyc[:], inv[:])

                osb = pool.tile([P, K], F)
                nc.vector.tensor_scalar_mul(osb[:], ex[:], nrm[:])
                nc.sync.dma_start(out[:, :], osb[:])
            else:
                # General path: log_k = a*x^2 + (b*y)*x + c*y^2.
                # Full-grid row sums on every core via iota y tiles.
                rs_tot = pool.tile([P, 1], F)
                for t in range(ntiles):
                    yt = pool.tile([P, 1], F, tag=f"yt{t}")
                    nc.gpsimd.iota(
                        yt[:], [[0, 1]], base=t * P - R, channel_multiplier=1,
                        allow_small_or_imprecise_dtypes=True,
                    )
                    ysqt = pool.tile([P, 1], F, tag=f"ysqt{t}")
                    nc.vector.tensor_mul(ysqt[:], yt[:], yt[:])
                    cyt = pool.tile([P, 1], F, tag=f"cyt{t}")
                    nc.scalar.mul(cyt[:], ysqt[:], c)
                    byt = pool.tile([P, 1], F, tag=f"byt{t}")
                    nc.scalar.mul(byt[:], yt[:], b)
                    v = pool.tile([P, K], F, tag=f"v{t}")
                    nc.vector.tensor_scalar_mul(v[:], xi[:], byt[:])
                    v2 = pool.tile([P, K], F, tag=f"v2{t}")
                    nc.vector.scalar_tensor_tensor(
                        v2[:], xsq[:], a, v[:],
                        op0=mybir.AluOpType.mult, op1=mybir.AluOpType.add,
                    )
                    et = pool.tile([P, K], F, tag=f"et{t}")
                    rst = pool.tile([P, 1], F, tag=f"rst{t}")
                    nc.scalar.activation(
                        et[:], v2[:], EXP, bias=cyt[:], accum_out=rst[:]
                    )
                    pad = ntiles * P - K
                    if t == ntiles - 1 and pad > 0:
                        nc.vector.memset(rst[P - pad :, :], 0.0)
                    if t == 0:
                        nc.vector.tensor_copy(rs_tot[:], rst[:])
                    else:
                        nc.vector.tensor_add(rs_tot[:], rs_tot[:], rst[:])
                # cross-partition total, broadcast to all partitions
                stot = pool.tile([P, 1], F)
                nc.gpsimd.partition_all_reduce(
                    stot[:], rs_tot[:], op=mybir.AluOpType.add
                )
                inv = pool.tile([P, 1], F)
                nc.vector.reciprocal(inv[:], stot[:])

                # this core's own rows from the ycoord input
                cy = pool.tile([P, 1], F)
                nc.scalar.mul(cy[:], ysq[:], c)
                by = pool.tile([P, 1], F)
                nc.scalar.mul(by[:], yc[:], b)
                v = pool.tile([P, K], F)
                nc.vector.tensor_scalar_mul(v[:], xi[:], by[:])
                v2 = pool.tile([P, K], F)
                nc.vector.scalar_tensor_tensor(
                    v2[:], xsq[:], a, v[:],
                    op0=mybir.AluOpType.mult, op1=mybir.AluOpType.add,
                )
                e = pool.tile([P, K], F)
                nc.scalar.activation(e[:], v2[:], EXP, bias=cy[:])
                osb = pool.tile([P, K], F)
                nc.vector.tensor_scalar_mul(osb[:], e[:], inv[:])
                nc.sync.dma_start(out[:, :], osb[:])

    nc.compile()
    return nc


LAST_RESULTS = None


def kernel(sigma1, sigma2, rho, kernel_size):
    _install_ntff_shim()
    from concourse.bass_utils import run_bass_kernel_spmd

    global LAST_RESULTS

    s1 = float(np.asarray(sigma1, dtype=np.float64).reshape(-1)[0])
    s2 = float(np.asarray(sigma2, dtype=np.float64).reshape(-1)[0])
    rv = float(np.asarray(rho, dtype=np.float64).reshape(-1)[0])
    K = int(np.asarray(kernel_size).reshape(-1)[0])
    R = K // 2
    ntiles = max(1, math.ceil(K / P))
    assert ntiles <= N_CORES, "kernel only supports K <= 1024"

    # launch constants (specialized per call; immediates in the kernel)
    a = float(np.float32(-1.0 / (2.0 * s1 * s1)))
    c = float(np.float32(-1.0 / (2.0 * s2 * s2)))
    b = float(np.float32(-rv / (s1 * s2)))
    use_rho = rv != 0.0

    nc = _build(a, c, b, K, ntiles, use_rho)

    in_maps = []
    for core in range(N_CORES):
        t = min(core, ntiles - 1)
        yvals = (np.arange(P, dtype=np.float32) + np.float32(t * P - R))[
            :, None
        ]
        in_maps.append({"ycoord": yvals})

    res = run_bass_kernel_spmd(nc, in_maps, core_ids=list(range(N_CORES)))
    LAST_RESULTS = res

    rows = np.vstack([res.results[t]["out"] for t in range(ntiles)])[:K]
    return rows.reshape(1, 1, K, K).astype(np.float32, copy=False)
